# revision 9
# baseline (speedup 1.0000x reference)
"""DeformConv2d forward on 8 Trainium2 NeuronCores (Bass/Tile).

x[8,128,96,96] f32, offset[8,18,96,96] f32, weight[128,128,3,3] f32
-> out[8,128,96,96] f32. Deformable 3x3 conv, pad 1, stride 1, bilinear
sampling with zero padding. Data-parallel over batch: one element per core.

Per-core pipeline (v3, position-major):
  A. x -> f16 into a zero-padded 98x98 image (SBUF, channel-major),
     PE-transposed to pixel-major x_cp[9728,256] f16 in DRAM, rows stored
     as vertical pairs (row j | row j+98) so one 1KB gather element covers
     the whole 2x2 bilinear quad.
  B. offsets PE-transposed to a position-packed layout.
  C. DVE index/weight math in [128, 9*72] packed layout: corner weights
     A0,A1,B0,B1 (f16) and padded row index jT=(y0c+1)*98+(x0c+1)
     (clamped in-range; out-of-range samples get zero weight). Corner
     weights are stored pair-duplicated in awD so the main-loop multiply
     can broadcast them along channels with a stride-0 AP dim while
     keeping DVE 2x mode.
  D. PE-transpose indices to row-major DRAM (16-partition-wrap-major),
     reload wrapped + replicate for the gather engine.
  F. Main loop per (chunk of 1536 positions, tap): two 768-index
     transpose=False gathers put 128 positions on partitions with the
     512-value quad per position in the free dim; one DVE multiply
     applies all 4 bilinear corner weights (per-position scalars);
     PE transpose-matmuls against identity accumulate the 4 weighted
     corners into PSUM, channel-major; Act evacuates to f16; one
     768-column GEMM per (tap, half) accumulates output over taps.
"""
import sys
if '/opt/trn_rl_repo' not in sys.path:
    sys.path.insert(0, '/opt/trn_rl_repo')

import numpy as np

import concourse.bacc as bacc_mod
import concourse.mybir as mybir
import concourse.tile as tile
from concourse.ap import AP

f32 = mybir.dt.float32
f16 = mybir.dt.float16
i16 = mybir.dt.int16
i32 = mybir.dt.int32
Alu = mybir.AluOpType

P = 128
H = W = 96
NPOS = H * W              # 9216
NT = NPOS // P            # 72 position tiles
K = 9
NF = K * NT               # 648
PW = 98                   # padded image row width
NTP = 76                  # padded-image transpose tiles (76*128 = 9728)
NPADR = NTP * P           # 9728 rows in x_tp
CW = 1536                 # main-loop position chunk
NCH = NPOS // CW          # 6 chunks
JB = CW // P              # 12 position blocks per chunk
GW = 768                  # per-gather index count (hw limit <= 896)


def _h(ap_or_handle):
    return ap_or_handle.tensor if hasattr(ap_or_handle, 'tensor') else ap_or_handle


def build_nc():
    nc = bacc_mod.Bacc(dynamic_dma_scratch_size=65536)
    x_in = nc.declare_dram_parameter("x", [P, NPOS], f32, isOutput=False)
    off_in = nc.declare_dram_parameter("offset", [18, NPOS], f32, isOutput=False)
    w_in = nc.declare_dram_parameter("weight", [P, 1152], f32, isOutput=False)
    out = nc.declare_dram_parameter("out", [P, NPOS], f16, isOutput=True)

    with tile.TileContext(nc) as tc:
        with tc.tile_pool(name="const", bufs=1) as cpool, \
             tc.tile_pool(name="persist", bufs=1) as ppool, \
             tc.tile_pool(name="dram", bufs=1, space="DRAM") as dpool:
            # x_cp[j] = [x_pad[row j], x_pad[row j+98]] (vertical pair):
            # one 1KB gather element covers the whole 2x2 bilinear quad.
            x_cp = dpool.tile([NPADR, 2 * P], f16, name="x_cp")
            idx_rows = dpool.tile([K, NPOS], i16, name="idx_rows")
            # ---------- constants ----------
            ident16 = cpool.tile([P, P], f16)
            ident32 = cpool.tile([P, P], f32)
            onesP = cpool.tile([P, P], f32)
            nc.vector.memset(onesP[:], 1.0)
            ramp128 = cpool.tile([P, P], f32)
            nc.vector.tensor_tensor_scan(ramp128[:], onesP[:], onesP[:], -1.0,
                                         Alu.mult, Alu.add)
            pcol_d = dpool.tile([1, P], f32, name="pcol_d")
            nc.sync.dma_start(pcol_d[:], ramp128[0:1, :])
            pcol = cpool.tile([P, 1], f32)
            src_p = AP(tensor=_h(pcol_d), offset=0, ap=[[1, P], [1, 1]])
            nc.sync.dma_start(pcol[:], src_p)
            nc.vector.tensor_scalar(ident32[:], ramp128[:], pcol[:], None,
                                    Alu.is_equal)
            nc.vector.tensor_copy(ident16[:], ident32[:])

            # ---------- persistent tiles ----------
            idxw = ppool.tile([P, K * 576], i16)
            WkT = ppool.tile([P, K * P], f16)
            # pair-duplicated corner weights: awD[p, ((k*NT+t)*4 + s)*2 + d]
            # = A_s[p, k*NT+t], slot order (TL, BL, TR, BR) = (A0, B0, A1, B1)
            awD = ppool.tile([P, NF * 8], f16)

            with tc.tile_pool(name="prepA", bufs=2) as pa:
                # extra P zero columns so the 98-shifted bottom transposes
                # of the last group read in-bounds zeros
                x16p = pa.tile([P, NPADR + P], f16, tag="x16p")
                w16 = pa.tile([P, 1152], f16, tag="w16")
                offt = pa.tile([P, NT * 18], f32, tag="offt")
                # ---- loads first (in-order DMA queue) ----
                with tc.tile_pool(name="ld", bufs=1) as pld, \
                     tc.tile_pool(name="psoP", bufs=2, space="PSUM") as psoP:
                    x_sb = pld.tile([P, NPOS], f32, tag="xsb")
                    nc.sync.dma_start(x_sb[:, 0:NPOS // 2],
                                      x_in[:, 0:NPOS // 2])
                    nc.sync.dma_start(x_sb[:, NPOS // 2:],
                                      x_in[:, NPOS // 2:])
                    off_sb = pld.tile([18, NPOS], f32, tag="offsb")
                    nc.sync.dma_start(off_sb[:], off_in[:])
                    w_sb = pld.tile([P, 1152], f32, tag="wsb")
                    nc.sync.dma_start(w_sb[:], w_in[:])

                    nc.gpsimd.memset(x16p[:], 0.0)
                    # interior: x16p[:, (y+1)*98 + (x+1)] = f16(x[:, y*96+x])
                    dst = x16p[:, PW:PW + H * PW].rearrange(
                        "p (r w) -> p r w", w=PW)[:, :, 1:1 + W]
                    srcx = x_sb[:].rearrange("p (r w) -> p r w", w=W)
                    nc.scalar.copy(dst[:, 0:H // 2, :], srcx[:, 0:H // 2, :])
                    nc.vector.tensor_copy(dst[:, H // 2:, :],
                                          srcx[:, H // 2:, :])

                    for tg in range(3):
                        pso = psoP.tile([P, 24 * 18], f32, tag="pso")
                        for j in range(24):
                            t = tg * 24 + j
                            nc.tensor.transpose(pso[:, j * 18:(j + 1) * 18],
                                                off_sb[0:18, t * P:(t + 1) * P],
                                                ident32[0:18, 0:18])
                        nc.scalar.copy(offt[:, tg * 432:(tg + 1) * 432], pso[:])

                    nc.scalar.copy(w16[:], w_sb[:])

                # ---- phase A: transpose padded image to pixel-major ----
                # Row j of x_cp = [pix j | pix j+98]: transpose each
                # 128-pixel group twice (once at +0, once at +98 columns)
                # so every x_cp row is built whole in SBUF and the DMA
                # writes contiguous 512B rows (no sub-512B penalty).
                with tc.tile_pool(name="ptP", bufs=4, space="PSUM") as ptP, \
                     tc.tile_pool(name="evpP", bufs=6) as evpP:
                    GB4 = 4  # store-groups per evac batch
                    for b in range(NTP // GB4):
                        ptb = ptP.tile([P, GB4 * 2 * P], f16, tag="ptb")
                        for g in range(GB4):
                            t = b * GB4 + g
                            nc.tensor.transpose(
                                ptb[:, (2 * g) * P:(2 * g + 1) * P],
                                x16p[:, t * P:(t + 1) * P], ident16[:])
                            nc.tensor.transpose(
                                ptb[:, (2 * g + 1) * P:(2 * g + 2) * P],
                                x16p[:, t * P + 98:t * P + 98 + P],
                                ident16[:])
                        evp = evpP.tile([P, GB4 * 2 * P], f16, tag="evp")
                        if b % 2 == 1:
                            nc.vector.tensor_copy(evp[:], ptb[:])
                        else:
                            nc.scalar.copy(evp[:], ptb[:])
                        dstc = AP(tensor=_h(x_cp),
                                  offset=b * GB4 * P * 2 * P,
                                  ap=[[2 * P, P], [P * 2 * P, GB4], [1, 2 * P]])
                        nc.sync.dma_start(
                            dstc,
                            evp[:].rearrange("r (g c) -> r g c", g=GB4))

                    # ---- conv weights -> WkT ----
                    for k in range(K):
                        wkc = pa.tile([P, P], f16, tag="wkc")
                        nc.scalar.copy(wkc[:], w16[:, k:1152:9])
                        ptw = ptP.tile([P, P], f16, tag="ptw")
                        nc.tensor.transpose(ptw[:], wkc[:], ident16[:])
                        nc.scalar.copy(WkT[:, k * P:(k + 1) * P], ptw[:])

                # ---- phase C: position/weight/index math (DVE) ----
                with tc.tile_pool(name="pc", bufs=1) as pc:
                    def st(tag, dt=f32):
                        return pc.tile([P, NT], dt, tag=tag, name=tag)

                    def mt(tag, dt=f32):
                        return pc.tile([P, NF], dt, tag=tag, name=tag)

                    posf = st("posf")
                    nc.vector.tensor_scalar(posf[:], ramp128[:, 0:NT], 128.0,
                                            None, Alu.mult)
                    nc.vector.tensor_scalar(posf[:], posf[:], pcol[:], None,
                                            Alu.add)
                    q0i = st("q0i", i32)
                    tmpq = st("tmpq")
                    nc.vector.tensor_scalar(tmpq[:], posf[:], 1.0 / 96.0, None,
                                            Alu.mult)
                    nc.vector.tensor_copy(q0i[:], tmpq[:])
                    q0 = st("q0")
                    nc.vector.tensor_copy(q0[:], q0i[:])
                    r0 = st("r0")
                    nc.vector.scalar_tensor_tensor(r0[:], q0[:], -96.0, posf[:],
                                                   Alu.mult, Alu.add)
                    ltz = st("ltz")
                    nc.vector.tensor_scalar(ltz[:], r0[:], 0.0, None, Alu.is_lt)
                    gez = st("gez")
                    nc.vector.tensor_scalar(gez[:], r0[:], 96.0, None, Alu.is_ge)
                    Rr = st("Rr")
                    nc.vector.tensor_tensor(Rr[:], q0[:], ltz[:], Alu.subtract)
                    nc.vector.tensor_tensor(Rr[:], Rr[:], gez[:], Alu.add)
                    Cc = st("Cc")
                    nc.vector.scalar_tensor_tensor(Cc[:], ltz[:], 96.0, r0[:],
                                                   Alu.mult, Alu.add)
                    nc.vector.scalar_tensor_tensor(Cc[:], gez[:], -96.0, Cc[:],
                                                   Alu.mult, Alu.add)

                    T1 = mt("T1")
                    T2 = mt("T2")
                    T3 = mt("T3")
                    T4 = mt("T4")
                    T5 = mt("T5")
                    T6 = mt("T6")
                    T7 = mt("T7")
                    T8 = mt("T8")
                    VI = mt("VI", i32)
                    A0 = mt("A0", f16)
                    A1 = mt("A1", f16)
                    B0 = mt("B0", f16)
                    B1 = mt("B1", f16)

                    for k in range(K):
                        ky, kx = k // 3, k % 3
                        nc.vector.tensor_scalar(T1[:, k * NT:(k + 1) * NT],
                                                Rr[:], float(ky - 1), None,
                                                Alu.add)
                        nc.vector.tensor_scalar(T2[:, k * NT:(k + 1) * NT],
                                                Cc[:], float(kx - 1), None,
                                                Alu.add)
                    offv = offt[:].rearrange("p (t pl) -> p pl t", pl=18)
                    # py (T1), px (T2)
                    nc.vector.tensor_tensor(
                        T1[:].rearrange("p (k t) -> p k t", k=K),
                        offv[:, 0:18:2, :],
                        T1[:].rearrange("p (k t) -> p k t", k=K), Alu.add)
                    nc.vector.tensor_tensor(
                        T2[:].rearrange("p (k t) -> p k t", k=K),
                        offv[:, 1:18:2, :],
                        T2[:].rearrange("p (k t) -> p k t", k=K), Alu.add)

                    def floor_frac(v, vf, fr, ng):
                        nc.vector.tensor_copy(VI[:], v[:])
                        nc.vector.tensor_copy(vf[:], VI[:])
                        nc.vector.tensor_tensor(fr[:], v[:], vf[:], Alu.subtract)
                        nc.vector.tensor_scalar(ng[:], fr[:], 0.0, None,
                                                Alu.is_lt)
                        nc.vector.tensor_tensor(vf[:], vf[:], ng[:],
                                                Alu.subtract)
                        nc.vector.tensor_tensor(fr[:], fr[:], ng[:], Alu.add)

                    floor_frac(T1, T3, T4, T7)   # y0=T3, fy=T4
                    floor_frac(T2, T5, T6, T7)   # x0=T5, fx=T6

                    # mask mm = (y0 in [-1,95]) & (x0 in [-1,95]) -> T8
                    nc.vector.tensor_scalar(T8[:], T3[:], -1.0, None, Alu.is_ge)
                    nc.vector.tensor_scalar(T1[:], T3[:], 95.0, None, Alu.is_le)
                    nc.vector.tensor_tensor(T8[:], T8[:], T1[:], Alu.mult)
                    nc.vector.tensor_scalar(T1[:], T5[:], -1.0, None, Alu.is_ge)
                    nc.vector.tensor_scalar(T2[:], T5[:], 95.0, None, Alu.is_le)
                    nc.vector.tensor_tensor(T1[:], T1[:], T2[:], Alu.mult)
                    nc.vector.tensor_tensor(T8[:], T8[:], T1[:], Alu.mult)

                    # wbot (T2) = fy*mm ; wtop (T1) = mm - wbot ; omfx (T7)
                    nc.vector.tensor_tensor(T2[:], T4[:], T8[:], Alu.mult)
                    nc.vector.tensor_tensor(T1[:], T8[:], T2[:], Alu.subtract)
                    nc.vector.tensor_scalar(T7[:], T6[:], -1.0, 1.0, Alu.mult,
                                            Alu.add)
                    nc.vector.tensor_tensor(A0[:], T1[:], T7[:], Alu.mult)
                    nc.vector.tensor_tensor(A1[:], T1[:], T6[:], Alu.mult)
                    nc.vector.tensor_tensor(B0[:], T2[:], T7[:], Alu.mult)
                    nc.vector.tensor_tensor(B1[:], T2[:], T6[:], Alu.mult)

                    # pair-duplicated slot-interleaved corner weights
                    for s, tt_ in enumerate((A0, B0, A1, B1)):
                        dstD = AP(tensor=_h(awD), offset=awD[:].offset + 2 * s,
                                  ap=[list(awD[:].ap[0]), [8, NF], [1, 2]])
                        srcD = AP(tensor=_h(tt_[:].tensor),
                                  offset=tt_[:].offset,
                                  ap=[list(tt_[:].ap[0]), [1, NF], [0, 2]])
                        nc.vector.tensor_copy(dstD, srcD)

                    # jT = (clip(y0)+1)*98 + clip(x0)+1 ; jB = jT + 98
                    nc.vector.tensor_scalar(T3[:], T3[:], -1.0, 95.0, Alu.max,
                                            Alu.min)
                    nc.vector.tensor_scalar(T5[:], T5[:], -1.0, 95.0, Alu.max,
                                            Alu.min)
                    nc.vector.tensor_scalar(T5[:], T5[:], 99.0, None, Alu.add)
                    nc.vector.scalar_tensor_tensor(T4[:], T3[:], 98.0, T5[:],
                                                   Alu.mult, Alu.add)   # JT

                    # ---- phase D: indices to wrap-major DRAM + reload ----
                    with tc.tile_pool(name="prepDp", bufs=2,
                                      space="PSUM") as pdp:
                        for k in range(K):
                            psi = pdp.tile([NT, P], f32, tag="psi")
                            nc.tensor.transpose(psi[:],
                                                T4[:, k * NT:(k + 1) * NT],
                                                ident32[:])
                            evi0 = pa.tile([NT, P], i16, tag="evi0")
                            nc.vector.tensor_copy(evi0[:], psi[:])
                            # pre-permute on DVE so the wrap-major store has
                            # contiguous runs: evi[c, el*8+eh] = evi0[c, e]
                            # with e = eh*16+el
                            evi = pa.tile([NT, P], i16, tag="evi")
                            nc.vector.tensor_copy(
                                evi[:].rearrange("c (el eh) -> c el eh",
                                                 el=16, eh=8),
                                evi0[:].rearrange("c (eh el) -> c el eh",
                                                  eh=8, el=16))
                            # idx i=t*128+eh*16+el sits at evi free f=el*8+eh;
                            # store to w = el*576 + 8t + eh (wrap-major)
                            dsti = AP(tensor=_h(idx_rows),
                                      offset=k * NPOS,
                                      ap=[[8, NT], [576, 16], [1, 8]])
                            srci = evi[:].rearrange(
                                "c (el eh) -> c el eh", el=16, eh=8)
                            nc.sync.dma_start(dsti, srci)

                            # wrapped reload; k=0 replicated eagerly so the
                            # first gather starts early, the rest in three
                            # wide DMAs after the last tap.
                            srcq = AP(tensor=_h(idx_rows), offset=k * NPOS,
                                      ap=[[576, 16], [1, 576]])
                            eng = nc.gpsimd if k == 0 else nc.sync
                            eng.dma_start(
                                idxw[0:16, k * 576:(k + 1) * 576], srcq)
                            if k == 0:
                                ks = slice(0, 576)
                                nc.sync.dma_start(idxw[16:32, ks],
                                                  idxw[0:16, ks])
                                nc.sync.dma_start(idxw[32:64, ks],
                                                  idxw[0:32, ks])
                                nc.sync.dma_start(idxw[64:128, ks],
                                                  idxw[0:64, ks])
                        ks = slice(576, K * 576)
                        nc.sync.dma_start(idxw[16:32, ks], idxw[0:16, ks])
                        nc.sync.dma_start(idxw[32:64, ks], idxw[0:32, ks])
                        nc.sync.dma_start(idxw[64:128, ks], idxw[0:64, ks])

            # ---------- phase F: main loop ----------
            xt_win = AP(tensor=_h(x_cp), offset=0,
                        ap=[[2 * P, NPADR - 1], [1, 4 * P]])
            with tc.tile_pool(name="g", bufs=3) as gp, \
                 tc.tile_pool(name="c4p", bufs=2) as c4p, \
                 tc.tile_pool(name="c4vp", bufs=3) as c4vp, \
                 tc.tile_pool(name="osp", bufs=2) as osp, \
                 tc.tile_pool(name="vpsP", bufs=2, space="PSUM") as vpsP, \
                 tc.tile_pool(name="outps", bufs=2, space="PSUM") as outps:
                iters = [(c, k) for c in range(NCH) for k in range(K)]

                def issue_gather(c, k):
                    # pos-major: out[p, jj, :] = quad of position
                    # (c*12 + h*6 + jj)*128 + p
                    g4 = gp.tile([P, JB, 4 * P], f16, tag="g4", name="g4")
                    for h in range(CW // GW):
                        i0 = k * 576 + (c * CW + h * GW) // 16
                        nc.gpsimd.dma_gather(
                            g4[:, h * 6:(h + 1) * 6, :], xt_win,
                            idxw[:, i0:i0 + GW // 16],
                            num_idxs=GW, num_idxs_reg=GW,
                            elem_size=4 * P, elem_step=2 * P, transpose=False)
                    return g4

                g4_next = issue_gather(*iters[0])
                for idx_it, (c, k) in enumerate(iters):
                    if k == 0:
                        out_ps = outps.tile([P, CW], f32, tag="ops",
                                            name="out_ps")
                    g4 = g4_next
                    if idx_it + 1 < len(iters):
                        g4_next = issue_gather(*iters[idx_it + 1])

                    # bilinear corner weighting: one DVE op, all 48 units;
                    # awD's stride-0 dim broadcasts each per-position weight
                    # along the 128 channels (last [1,2] dim keeps 2x mode)
                    c4 = c4p.tile([P, JB * 4, P], f16, tag="c4", name="c4")
                    aw_ap = AP(tensor=_h(awD),
                               offset=awD[:].offset + (k * NT + c * JB) * 8,
                               ap=[list(awD[:].ap[0]), [2, JB * 4],
                                   [0, P // 2], [1, 2]])
                    nc.vector.tensor_tensor(
                        c4[:], g4[:].rearrange("p j (s c) -> p (j s) c", c=P),
                        aw_ap, Alu.mult)

                    HB = 512  # one f32 PSUM bank; matmul out can't cross banks
                    for h in range(CW // HB):
                        vps = vpsP.tile([P, HB], f32, tag="vps")
                        for jj in range(HB // P):
                            u = (h * (HB // P) + jj) * 4
                            for s in range(4):
                                nc.tensor.matmul(
                                    vps[:, jj * P:(jj + 1) * P],
                                    c4[:, u + s, :], ident16[:],
                                    start=(s == 0), stop=(s == 3),
                                    skip_group_check=True)
                        c4v = c4vp.tile([P, HB], f16, tag="c4v")
                        nc.scalar.copy(c4v[:], vps[:])
                        nc.tensor.matmul(
                            out_ps[:, h * HB:(h + 1) * HB],
                            WkT[:, k * P:(k + 1) * P], c4v[:],
                            start=(k == 0), stop=(k == K - 1),
                            skip_group_check=True)
                        if k == K - 1:
                            # store each 512-block as soon as its
                            # accumulation group stops (short drain tail)
                            osb = osp.tile([P, HB], f16, tag="osb")
                            nc.scalar.copy(osb[:],
                                           out_ps[:, h * HB:(h + 1) * HB])
                            nc.sync.dma_start(
                                out[:, c * CW + h * HB:c * CW + (h + 1) * HB],
                                osb[:])
    nc.compile()
    return nc


_NC = None


def kernel(x, offset, weight):
    global _NC
    if _NC is None:
        _NC = build_nc()
    from concourse.bass_utils import run_bass_kernel_spmd
    B = x.shape[0]
    w2 = np.ascontiguousarray(np.asarray(weight).reshape(P, 1152)).astype(np.float32)
    in_maps = []
    for b in range(B):
        in_maps.append({
            "x": np.ascontiguousarray(np.asarray(x)[b].reshape(P, NPOS), dtype=np.float32),
            "offset": np.ascontiguousarray(np.asarray(offset)[b].reshape(18, NPOS), dtype=np.float32),
            "weight": w2,
        })
    res = run_bass_kernel_spmd(_NC, in_maps, list(range(B)))
    outs = [res.results[b]["out"].reshape(P, H, W) for b in range(B)]
    return np.stack(outs).astype(np.float32)


# revision 15
# speedup vs baseline: 1.0141x; 1.0141x over previous
"""DeformConv2d forward on 8 Trainium2 NeuronCores (Bass/Tile).

x[8,128,96,96] f32, offset[8,18,96,96] f32, weight[128,128,3,3] f32
-> out[8,128,96,96] f32. Deformable 3x3 conv, pad 1, stride 1, bilinear
sampling with zero padding. Data-parallel over batch: one element per core.

Per-core pipeline (v3, position-major):
  A. x -> f16 into a zero-padded 98x98 image (SBUF, channel-major),
     PE-transposed to pixel-major x_cp[9728,256] f16 in DRAM, rows stored
     as vertical pairs (row j | row j+98) so one 1KB gather element covers
     the whole 2x2 bilinear quad.
  B. offsets PE-transposed to a position-packed layout.
  C. DVE index/weight math in [128, 9*72] packed layout: corner weights
     A0,A1,B0,B1 (f16) and padded row index jT=(y0c+1)*98+(x0c+1)
     (clamped in-range; out-of-range samples get zero weight). Corner
     weights are stored pair-duplicated in awD so the main-loop multiply
     can broadcast them along channels with a stride-0 AP dim while
     keeping DVE 2x mode.
  D. PE-transpose indices to row-major DRAM (16-partition-wrap-major),
     reload wrapped + replicate for the gather engine.
  F. Main loop per (chunk of 1536 positions, tap): two 768-index
     transpose=False gathers put 128 positions on partitions with the
     512-value quad per position in the free dim; one DVE multiply
     applies all 4 bilinear corner weights (per-position scalars);
     PE transpose-matmuls against identity accumulate the 4 weighted
     corners into PSUM, channel-major; Act evacuates to f16; one
     768-column GEMM per (tap, half) accumulates output over taps.
"""
import sys
if '/opt/trn_rl_repo' not in sys.path:
    sys.path.insert(0, '/opt/trn_rl_repo')

import numpy as np

import concourse.bacc as bacc_mod
import concourse.mybir as mybir
import concourse.tile as tile
from concourse.ap import AP

f32 = mybir.dt.float32
f16 = mybir.dt.float16
i16 = mybir.dt.int16
i32 = mybir.dt.int32
Alu = mybir.AluOpType

P = 128
H = W = 96
NPOS = H * W              # 9216
NT = NPOS // P            # 72 position tiles
K = 9
NF = K * NT               # 648
PW = 98                   # padded image row width
NTP = 76                  # padded-image transpose tiles (76*128 = 9728)
NPADR = NTP * P           # 9728 rows in x_tp
CW = 1536                 # main-loop position chunk
NCH = NPOS // CW          # 6 chunks
JB = CW // P              # 12 position blocks per chunk
GW = 768                  # per-gather index count (hw limit <= 896)


def _h(ap_or_handle):
    return ap_or_handle.tensor if hasattr(ap_or_handle, 'tensor') else ap_or_handle


def build_nc():
    nc = bacc_mod.Bacc(dynamic_dma_scratch_size=65536)
    x_in = nc.declare_dram_parameter("x", [P, NPOS], f32, isOutput=False)
    off_in = nc.declare_dram_parameter("offset", [18, NPOS], f32, isOutput=False)
    w_in = nc.declare_dram_parameter("weight", [P, 1152], f32, isOutput=False)
    out = nc.declare_dram_parameter("out", [P, NPOS], f16, isOutput=True)

    with tile.TileContext(nc) as tc:
        with tc.tile_pool(name="const", bufs=1) as cpool, \
             tc.tile_pool(name="persist", bufs=1) as ppool, \
             tc.tile_pool(name="dram", bufs=1, space="DRAM") as dpool:
            # x_cp[j] = [x_pad[row j], x_pad[row j+98]] (vertical pair):
            # one 1KB gather element covers the whole 2x2 bilinear quad.
            x_cp = dpool.tile([NPADR, 2 * P], f16, name="x_cp")
            idx_rows = dpool.tile([K, NPOS], i16, name="idx_rows")
            # ---------- constants ----------
            ident16 = cpool.tile([P, P], f16)
            ident32 = cpool.tile([P, P], f32)
            onesP = cpool.tile([P, P], f32)
            nc.vector.memset(onesP[:], 1.0)
            ramp128 = cpool.tile([P, P], f32)
            nc.vector.tensor_tensor_scan(ramp128[:], onesP[:], onesP[:], -1.0,
                                         Alu.mult, Alu.add)
            pcol_d = dpool.tile([1, P], f32, name="pcol_d")
            nc.sync.dma_start(pcol_d[:], ramp128[0:1, :])
            pcol = cpool.tile([P, 1], f32)
            src_p = AP(tensor=_h(pcol_d), offset=0, ap=[[1, P], [1, 1]])
            nc.sync.dma_start(pcol[:], src_p)
            nc.vector.tensor_scalar(ident32[:], ramp128[:], pcol[:], None,
                                    Alu.is_equal)
            nc.vector.tensor_copy(ident16[:], ident32[:])

            # ---------- persistent tiles ----------
            # one idx tile per tap so a gather only waits on its own tap's
            # index pipeline (deps are tile-granular)
            idxw = [ppool.tile([P, 576], i16, name=f"idxw{k}")
                    for k in range(K)]
            WkT = ppool.tile([P, K * P], f16)
            # pair-duplicated corner weights: awD[p, ((k*NT+t)*4 + s)*2 + d]
            # = A_s[p, k*NT+t], slot order (TL, BL, TR, BR) = (A0, B0, A1, B1)
            awD = ppool.tile([P, NF * 8], f16)

            with tc.tile_pool(name="prepA", bufs=2) as pa:
                # extra P zero columns so the 98-shifted bottom transposes
                # of the last group read in-bounds zeros
                x16p = pa.tile([P, NPADR + P], f16, tag="x16p")
                w16 = pa.tile([P, 1152], f16, tag="w16")
                offt = pa.tile([P, NT * 18], f32, tag="offt")
                # ---- loads first (in-order DMA queue) ----
                with tc.tile_pool(name="ld", bufs=1) as pld, \
                     tc.tile_pool(name="psoP", bufs=2, space="PSUM") as psoP:
                    x_sb = pld.tile([P, NPOS], f32, tag="xsb")
                    nc.sync.dma_start(x_sb[:, 0:NPOS // 2],
                                      x_in[:, 0:NPOS // 2])
                    nc.sync.dma_start(x_sb[:, NPOS // 2:],
                                      x_in[:, NPOS // 2:])
                    off_sb = pld.tile([18, NPOS], f32, tag="offsb")
                    nc.sync.dma_start(off_sb[:], off_in[:])
                    w_sb = pld.tile([P, 1152], f32, tag="wsb")
                    nc.sync.dma_start(w_sb[:], w_in[:])

                    nc.gpsimd.memset(x16p[:], 0.0)
                    # interior: x16p[:, (y+1)*98 + (x+1)] = f16(x[:, y*96+x])
                    # (all on Act: DVE is reserved for the phase C chain)
                    dst = x16p[:, PW:PW + H * PW].rearrange(
                        "p (r w) -> p r w", w=PW)[:, :, 1:1 + W]
                    srcx = x_sb[:].rearrange("p (r w) -> p r w", w=W)
                    nc.scalar.copy(dst[:, 0:H // 2, :], srcx[:, 0:H // 2, :])
                    nc.scalar.copy(dst[:, H // 2:, :], srcx[:, H // 2:, :])

                    for tg in range(3):
                        pso = psoP.tile([P, 24 * 18], f32, tag="pso")
                        for j in range(24):
                            t = tg * 24 + j
                            nc.tensor.transpose(pso[:, j * 18:(j + 1) * 18],
                                                off_sb[0:18, t * P:(t + 1) * P],
                                                ident32[0:18, 0:18])
                        nc.scalar.copy(offt[:, tg * 432:(tg + 1) * 432], pso[:])

                    nc.scalar.copy(w16[:], w_sb[:])

                # ---- phase A: transpose padded image to pixel-major ----
                # Row j of x_cp = [pix j | pix j+98]: transpose each
                # 128-pixel group twice (once at +0, once at +98 columns)
                # so every x_cp row is built whole in SBUF and the DMA
                # writes contiguous 512B rows (no sub-512B penalty).
                with tc.tile_pool(name="ptP", bufs=4, space="PSUM") as ptP, \
                     tc.tile_pool(name="evpP", bufs=6) as evpP:
                    GB4 = 4  # store-groups per evac batch
                    for b in range(NTP // GB4):
                        ptb = ptP.tile([P, GB4 * 2 * P], f16, tag="ptb")
                        for g in range(GB4):
                            t = b * GB4 + g
                            nc.tensor.transpose(
                                ptb[:, (2 * g) * P:(2 * g + 1) * P],
                                x16p[:, t * P:(t + 1) * P], ident16[:])
                            nc.tensor.transpose(
                                ptb[:, (2 * g + 1) * P:(2 * g + 2) * P],
                                x16p[:, t * P + 98:t * P + 98 + P],
                                ident16[:])
                        evp = evpP.tile([P, GB4 * 2 * P], f16, tag="evp")
                        nc.scalar.copy(evp[:], ptb[:])
                        dstc = AP(tensor=_h(x_cp),
                                  offset=b * GB4 * P * 2 * P,
                                  ap=[[2 * P, P], [P * 2 * P, GB4], [1, 2 * P]])
                        nc.sync.dma_start(
                            dstc,
                            evp[:].rearrange("r (g c) -> r g c", g=GB4))

                    # ---- conv weights -> WkT ----
                    for k in range(K):
                        wkc = pa.tile([P, P], f16, tag="wkc")
                        nc.scalar.copy(wkc[:], w16[:, k:1152:9])
                        ptw = ptP.tile([P, P], f16, tag="ptw")
                        nc.tensor.transpose(ptw[:], wkc[:], ident16[:])
                        nc.scalar.copy(WkT[:, k * P:(k + 1) * P], ptw[:])

                # ---- phase C: position/weight/index math (DVE) ----
                with tc.tile_pool(name="pc", bufs=1) as pc:
                    def st(tag, dt=f32):
                        return pc.tile([P, NT], dt, tag=tag, name=tag)

                    def mt(tag, dt=f32):
                        return pc.tile([P, NF], dt, tag=tag, name=tag)

                    posf = st("posf")
                    nc.vector.tensor_scalar(posf[:], ramp128[:, 0:NT], 128.0,
                                            None, Alu.mult)
                    nc.vector.tensor_scalar(posf[:], posf[:], pcol[:], None,
                                            Alu.add)
                    q0i = st("q0i", i32)
                    tmpq = st("tmpq")
                    nc.vector.tensor_scalar(tmpq[:], posf[:], 1.0 / 96.0, None,
                                            Alu.mult)
                    nc.vector.tensor_copy(q0i[:], tmpq[:])
                    q0 = st("q0")
                    nc.vector.tensor_copy(q0[:], q0i[:])
                    r0 = st("r0")
                    nc.vector.scalar_tensor_tensor(r0[:], q0[:], -96.0, posf[:],
                                                   Alu.mult, Alu.add)
                    ltz = st("ltz")
                    nc.vector.tensor_scalar(ltz[:], r0[:], 0.0, None, Alu.is_lt)
                    gez = st("gez")
                    nc.vector.tensor_scalar(gez[:], r0[:], 96.0, None, Alu.is_ge)
                    Rr = st("Rr")
                    nc.vector.tensor_tensor(Rr[:], q0[:], ltz[:], Alu.subtract)
                    nc.vector.tensor_tensor(Rr[:], Rr[:], gez[:], Alu.add)
                    Cc = st("Cc")
                    nc.vector.scalar_tensor_tensor(Cc[:], ltz[:], 96.0, r0[:],
                                                   Alu.mult, Alu.add)
                    nc.vector.scalar_tensor_tensor(Cc[:], gez[:], -96.0, Cc[:],
                                                   Alu.mult, Alu.add)

                    T1 = mt("T1")
                    T2 = mt("T2")
                    T3 = mt("T3")
                    T4 = mt("T4")
                    T5 = mt("T5")
                    T6 = mt("T6")
                    T7 = mt("T7")
                    T8 = mt("T8")
                    VI = mt("VI", i32)
                    A0 = mt("A0", f16)
                    A1 = mt("A1", f16)
                    B0 = mt("B0", f16)
                    B1 = mt("B1", f16)

                    for k in range(K):
                        ky, kx = k // 3, k % 3
                        nc.vector.tensor_scalar(T1[:, k * NT:(k + 1) * NT],
                                                Rr[:], float(ky - 1), None,
                                                Alu.add)
                        nc.vector.tensor_scalar(T2[:, k * NT:(k + 1) * NT],
                                                Cc[:], float(kx - 1), None,
                                                Alu.add)
                    offv = offt[:].rearrange("p (t pl) -> p pl t", pl=18)
                    # py (T1), px (T2)
                    nc.vector.tensor_tensor(
                        T1[:].rearrange("p (k t) -> p k t", k=K),
                        offv[:, 0:18:2, :],
                        T1[:].rearrange("p (k t) -> p k t", k=K), Alu.add)
                    nc.vector.tensor_tensor(
                        T2[:].rearrange("p (k t) -> p k t", k=K),
                        offv[:, 1:18:2, :],
                        T2[:].rearrange("p (k t) -> p k t", k=K), Alu.add)

                    def floor_frac(v, vf, fr, ng):
                        nc.vector.tensor_copy(VI[:], v[:])
                        nc.vector.tensor_copy(vf[:], VI[:])
                        nc.vector.tensor_tensor(fr[:], v[:], vf[:], Alu.subtract)
                        nc.vector.tensor_scalar(ng[:], fr[:], 0.0, None,
                                                Alu.is_lt)
                        nc.vector.tensor_tensor(vf[:], vf[:], ng[:],
                                                Alu.subtract)
                        nc.vector.tensor_tensor(fr[:], fr[:], ng[:], Alu.add)

                    floor_frac(T1, T3, T4, T7)   # y0=T3, fy=T4
                    floor_frac(T2, T5, T6, T7)   # x0=T5, fx=T6

                    # jT first (so the idx pipeline and the first gathers
                    # start while the corner weights are still being built):
                    # jT = (clip(y0)+1)*98 + clip(x0)+1
                    T3c = mt("T3c")
                    T5c = mt("T5c")
                    TJ = mt("TJ")
                    nc.vector.tensor_scalar(T3c[:], T3[:], -1.0, 95.0, Alu.max,
                                            Alu.min)
                    nc.vector.tensor_scalar(T5c[:], T5[:], -1.0, 95.0, Alu.max,
                                            Alu.min)
                    nc.vector.tensor_scalar(T5c[:], T5c[:], 99.0, None, Alu.add)
                    nc.vector.scalar_tensor_tensor(TJ[:], T3c[:], 98.0, T5c[:],
                                                   Alu.mult, Alu.add)   # JT

                    # ---- phase D: indices to wrap-major DRAM + reload ----
                    with tc.tile_pool(name="prepDp", bufs=2,
                                      space="PSUM") as pdp:
                        for k in range(K):
                            psi = pdp.tile([NT, P], f32, tag="psi")
                            nc.tensor.transpose(psi[:],
                                                TJ[:, k * NT:(k + 1) * NT],
                                                ident32[:])
                            evi0 = pa.tile([NT, P], i16, tag="evi0")
                            nc.vector.tensor_copy(evi0[:], psi[:])
                            # pre-permute on DVE so the wrap-major store has
                            # contiguous runs: evi[c, el*8+eh] = evi0[c, e]
                            # with e = eh*16+el
                            evi = pa.tile([NT, P], i16, tag="evi")
                            nc.vector.tensor_copy(
                                evi[:].rearrange("c (el eh) -> c el eh",
                                                 el=16, eh=8),
                                evi0[:].rearrange("c (eh el) -> c el eh",
                                                  eh=8, el=16))
                            # idx i=t*128+eh*16+el sits at evi free f=el*8+eh;
                            # store to w = el*576 + 8t + eh (wrap-major)
                            dsti = AP(tensor=_h(idx_rows),
                                      offset=k * NPOS,
                                      ap=[[8, NT], [576, 16], [1, 8]])
                            srci = evi[:].rearrange(
                                "c (el eh) -> c el eh", el=16, eh=8)
                            nc.sync.dma_start(dsti, srci)

                            # wrapped reload + replicate to 128 partitions,
                            # per-tap tile so gather k waits only on tap k
                            srcq = AP(tensor=_h(idx_rows), offset=k * NPOS,
                                      ap=[[576, 16], [1, 576]])
                            eng = nc.gpsimd if k == 0 else nc.sync
                            eng.dma_start(idxw[k][0:16, :], srcq)
                            nc.sync.dma_start(idxw[k][16:32, :],
                                              idxw[k][0:16, :])
                            nc.sync.dma_start(idxw[k][32:64, :],
                                              idxw[k][0:32, :])
                            nc.sync.dma_start(idxw[k][64:128, :],
                                              idxw[k][0:64, :])

                    # mask mm = (y0 in [-1,95]) & (x0 in [-1,95]) -> T8
                    nc.vector.tensor_scalar(T8[:], T3[:], -1.0, None, Alu.is_ge)
                    nc.vector.tensor_scalar(T1[:], T3[:], 95.0, None, Alu.is_le)
                    nc.vector.tensor_tensor(T8[:], T8[:], T1[:], Alu.mult)
                    nc.vector.tensor_scalar(T1[:], T5[:], -1.0, None, Alu.is_ge)
                    nc.vector.tensor_scalar(T2[:], T5[:], 95.0, None, Alu.is_le)
                    nc.vector.tensor_tensor(T1[:], T1[:], T2[:], Alu.mult)
                    nc.vector.tensor_tensor(T8[:], T8[:], T1[:], Alu.mult)

                    # wbot (T2) = fy*mm ; wtop (T1) = mm - wbot ; omfx (T7)
                    nc.vector.tensor_tensor(T2[:], T4[:], T8[:], Alu.mult)
                    nc.vector.tensor_tensor(T1[:], T8[:], T2[:], Alu.subtract)
                    nc.vector.tensor_scalar(T7[:], T6[:], -1.0, 1.0, Alu.mult,
                                            Alu.add)
                    nc.vector.tensor_tensor(A0[:], T1[:], T7[:], Alu.mult)
                    nc.vector.tensor_tensor(A1[:], T1[:], T6[:], Alu.mult)
                    nc.vector.tensor_tensor(B0[:], T2[:], T7[:], Alu.mult)
                    nc.vector.tensor_tensor(B1[:], T2[:], T6[:], Alu.mult)

                    # pair-duplicated slot-interleaved corner weights
                    for s, tt_ in enumerate((A0, B0, A1, B1)):
                        dstD = AP(tensor=_h(awD), offset=awD[:].offset + 2 * s,
                                  ap=[list(awD[:].ap[0]), [8, NF], [1, 2]])
                        srcD = AP(tensor=_h(tt_[:].tensor),
                                  offset=tt_[:].offset,
                                  ap=[list(tt_[:].ap[0]), [1, NF], [0, 2]])
                        nc.vector.tensor_copy(dstD, srcD)

            # ---------- phase F: main loop ----------
            xt_win = AP(tensor=_h(x_cp), offset=0,
                        ap=[[2 * P, NPADR - 1], [1, 4 * P]])
            with tc.tile_pool(name="g", bufs=3) as gp, \
                 tc.tile_pool(name="c4p", bufs=2) as c4p, \
                 tc.tile_pool(name="c4vp", bufs=3) as c4vp, \
                 tc.tile_pool(name="osp", bufs=2) as osp, \
                 tc.tile_pool(name="vpsP", bufs=2, space="PSUM") as vpsP, \
                 tc.tile_pool(name="outps", bufs=2, space="PSUM") as outps:
                iters = [(c, k) for c in range(NCH) for k in range(K)]

                def issue_gather(c, k):
                    # pos-major: out[p, jj, :] = quad of position
                    # (c*12 + h*6 + jj)*128 + p
                    g4 = gp.tile([P, JB, 4 * P], f16, tag="g4", name="g4")
                    for h in range(CW // GW):
                        i0 = (c * CW + h * GW) // 16
                        nc.gpsimd.dma_gather(
                            g4[:, h * 6:(h + 1) * 6, :], xt_win,
                            idxw[k][:, i0:i0 + GW // 16],
                            num_idxs=GW, num_idxs_reg=GW,
                            elem_size=4 * P, elem_step=2 * P, transpose=False)
                    return g4

                g4_next = issue_gather(*iters[0])
                for idx_it, (c, k) in enumerate(iters):
                    if k == 0:
                        out_ps = outps.tile([P, CW], f32, tag="ops",
                                            name="out_ps")
                    g4 = g4_next
                    if idx_it + 1 < len(iters):
                        g4_next = issue_gather(*iters[idx_it + 1])

                    # bilinear corner weighting: one DVE op, all 48 units;
                    # awD's stride-0 dim broadcasts each per-position weight
                    # along the 128 channels (last [1,2] dim keeps 2x mode)
                    c4 = c4p.tile([P, JB * 4, P], f16, tag="c4", name="c4")
                    aw_ap = AP(tensor=_h(awD),
                               offset=awD[:].offset + (k * NT + c * JB) * 8,
                               ap=[list(awD[:].ap[0]), [2, JB * 4],
                                   [0, P // 2], [1, 2]])
                    nc.vector.tensor_tensor(
                        c4[:], g4[:].rearrange("p j (s c) -> p (j s) c", c=P),
                        aw_ap, Alu.mult)

                    HB = 512  # one f32 PSUM bank; matmul out can't cross banks
                    for h in range(CW // HB):
                        vps = vpsP.tile([P, HB], f32, tag="vps")
                        for jj in range(HB // P):
                            u = (h * (HB // P) + jj) * 4
                            for s in range(4):
                                nc.tensor.matmul(
                                    vps[:, jj * P:(jj + 1) * P],
                                    c4[:, u + s, :], ident16[:],
                                    start=(s == 0), stop=(s == 3),
                                    skip_group_check=True)
                        c4v = c4vp.tile([P, HB], f16, tag="c4v")
                        nc.scalar.copy(c4v[:], vps[:])
                        nc.tensor.matmul(
                            out_ps[:, h * HB:(h + 1) * HB],
                            WkT[:, k * P:(k + 1) * P], c4v[:],
                            start=(k == 0), stop=(k == K - 1),
                            skip_group_check=True)
                        if k == K - 1:
                            # store each 512-block as soon as its
                            # accumulation group stops (short drain tail)
                            osb = osp.tile([P, HB], f16, tag="osb")
                            nc.scalar.copy(osb[:],
                                           out_ps[:, h * HB:(h + 1) * HB])
                            nc.sync.dma_start(
                                out[:, c * CW + h * HB:c * CW + (h + 1) * HB],
                                osb[:])
    nc.compile()
    return nc


_NC = None


def kernel(x, offset, weight):
    global _NC
    if _NC is None:
        _NC = build_nc()
    from concourse.bass_utils import run_bass_kernel_spmd
    B = x.shape[0]
    w2 = np.ascontiguousarray(np.asarray(weight).reshape(P, 1152)).astype(np.float32)
    in_maps = []
    for b in range(B):
        in_maps.append({
            "x": np.ascontiguousarray(np.asarray(x)[b].reshape(P, NPOS), dtype=np.float32),
            "offset": np.ascontiguousarray(np.asarray(offset)[b].reshape(18, NPOS), dtype=np.float32),
            "weight": w2,
        })
    res = run_bass_kernel_spmd(_NC, in_maps, list(range(B)))
    outs = [res.results[b]["out"].reshape(P, H, W) for b in range(B)]
    return np.stack(outs).astype(np.float32)


# revision 19
# speedup vs baseline: 1.1288x; 1.1131x over previous
"""DeformConv2d forward on 8 Trainium2 NeuronCores (Bass/Tile).

x[8,128,96,96] f32, offset[8,18,96,96] f32, weight[128,128,3,3] f32
-> out[8,128,96,96] f32. Deformable 3x3 conv, pad 1, stride 1, bilinear
sampling with zero padding. Data-parallel over batch: one element per core.

Per-core pipeline (v3, position-major):
  A. x -> f16 into a zero-padded 98x98 image (SBUF, channel-major),
     PE-transposed to pixel-major x_cp[9728,256] f16 in DRAM, rows stored
     as vertical pairs (row j | row j+98) so one 1KB gather element covers
     the whole 2x2 bilinear quad.
  B. offsets PE-transposed to a position-packed layout.
  C. DVE index/weight math in [128, 9*72] packed layout: corner weights
     A0,A1,B0,B1 (f16) and padded row index jT=(y0c+1)*98+(x0c+1)
     (clamped in-range; out-of-range samples get zero weight). Corner
     weights are stored pair-duplicated in awD so the main-loop multiply
     can broadcast them along channels with a stride-0 AP dim while
     keeping DVE 2x mode.
  D. PE-transpose indices to row-major DRAM (16-partition-wrap-major),
     reload wrapped + replicate for the gather engine.
  F. Main loop per (chunk of 1536 positions, tap): two 768-index
     transpose=False gathers put 128 positions on partitions with the
     512-value quad per position in the free dim; one DVE multiply
     applies all 4 bilinear corner weights (per-position scalars);
     PE transpose-matmuls against identity accumulate the 4 weighted
     corners into PSUM, channel-major; Act evacuates to f16; one
     768-column GEMM per (tap, half) accumulates output over taps.
"""
import sys
if '/opt/trn_rl_repo' not in sys.path:
    sys.path.insert(0, '/opt/trn_rl_repo')

import numpy as np

import concourse.bacc as bacc_mod
import concourse.mybir as mybir
import concourse.tile as tile
from concourse.ap import AP

f32 = mybir.dt.float32
f16 = mybir.dt.float16
i16 = mybir.dt.int16
i32 = mybir.dt.int32
Alu = mybir.AluOpType

P = 128
H = W = 96
NPOS = H * W              # 9216
NT = NPOS // P            # 72 position tiles
K = 9
NF = K * NT               # 648
PW = 98                   # padded image row width
NTP = 76                  # padded-image transpose tiles (76*128 = 9728)
NPADR = NTP * P           # 9728 rows in x_tp
CW = 1536                 # main-loop position chunk
NCH = NPOS // CW          # 6 chunks
JB = CW // P              # 12 position blocks per chunk
GW = 768                  # per-gather index count (hw limit <= 896)


def _h(ap_or_handle):
    return ap_or_handle.tensor if hasattr(ap_or_handle, 'tensor') else ap_or_handle


def build_nc():
    nc = bacc_mod.Bacc(dynamic_dma_scratch_size=65536)
    x_in = nc.declare_dram_parameter("x", [P, NPOS], f32, isOutput=False)
    off_in = nc.declare_dram_parameter("offset", [18, NPOS], f32, isOutput=False)
    w_in = nc.declare_dram_parameter("weight", [P, 1152], f32, isOutput=False)
    out = nc.declare_dram_parameter("out", [P, NPOS], f16, isOutput=True)

    with tile.TileContext(nc) as tc:
        with tc.tile_pool(name="const", bufs=1) as cpool, \
             tc.tile_pool(name="persist", bufs=1) as ppool, \
             tc.tile_pool(name="dram", bufs=1, space="DRAM") as dpool:
            # x_cp[j] = [x_pad[row j], x_pad[row j+98]] (vertical pair):
            # one 1KB gather element covers the whole 2x2 bilinear quad.
            x_cp = dpool.tile([NPADR, 2 * P], f16, name="x_cp")
            idx_rows = dpool.tile([K, NPOS], i16, name="idx_rows")
            # ---------- constants ----------
            ident16 = cpool.tile([P, P], f16)
            ident32 = cpool.tile([P, P], f32)
            onesP = cpool.tile([P, P], f32)
            nc.vector.memset(onesP[:], 1.0)
            ramp128 = cpool.tile([P, P], f32)
            nc.vector.tensor_tensor_scan(ramp128[:], onesP[:], onesP[:], -1.0,
                                         Alu.mult, Alu.add)
            pcol_d = dpool.tile([1, P], f32, name="pcol_d")
            nc.sync.dma_start(pcol_d[:], ramp128[0:1, :])
            pcol = cpool.tile([P, 1], f32)
            src_p = AP(tensor=_h(pcol_d), offset=0, ap=[[1, P], [1, 1]])
            nc.sync.dma_start(pcol[:], src_p)
            nc.vector.tensor_scalar(ident32[:], ramp128[:], pcol[:], None,
                                    Alu.is_equal)
            nc.vector.tensor_copy(ident16[:], ident32[:])

            # ---------- persistent tiles ----------
            # one idx tile per tap so a gather only waits on its own tap's
            # index pipeline (deps are tile-granular)
            idxw = [ppool.tile([P, 576], i16, name=f"idxw{k}")
                    for k in range(K)]
            WkT = ppool.tile([P, K * P], f16)
            # pair-duplicated corner weights: awD[p, ((k*NT+t)*4 + s)*2 + d]
            # = A_s[p, k*NT+t], slot order (TL, BL, TR, BR) = (A0, B0, A1, B1)
            awD = ppool.tile([P, NF * 8], f16)

            with tc.tile_pool(name="pc", bufs=1) as pc, \
                 tc.tile_pool(name="prepA", bufs=1) as pa, \
                 tc.tile_pool(name="evip", bufs=2) as evip, \
                 tc.tile_pool(name="ld", bufs=1) as pld, \
                 tc.tile_pool(name="evpP", bufs=6) as evpP, \
                 tc.tile_pool(name="psoP", bufs=2, space="PSUM") as psoP, \
                 tc.tile_pool(name="ptP", bufs=3, space="PSUM") as ptP, \
                 tc.tile_pool(name="ptwP", bufs=1, space="PSUM") as ptwP, \
                 tc.tile_pool(name="prepDp", bufs=2, space="PSUM") as pdp:
                # extra P zero columns so the 98-shifted bottom transposes
                # of the last group read in-bounds zeros
                x16p = pa.tile([P, NPADR + P], f16, tag="x16p")
                w16 = pa.tile([P, 1152], f16, tag="w16")
                offt = pa.tile([P, NT * 18], f32, tag="offt")

                # ---- offset first: it gates the DVE index chain ----
                nc.gpsimd.memset(x16p[:], 0.0)
                for tg in range(3):
                    off_sb = pld.tile([18, NPOS // 3], f32,
                                      tag=f"offsb{tg % 2}")
                    nc.sync.dma_start(off_sb[:],
                                      off_in[:, tg * 3072:(tg + 1) * 3072])
                    pso = psoP.tile([P, 24 * 18], f32, tag="pso")
                    for j in range(24):
                        nc.tensor.transpose(pso[:, j * 18:(j + 1) * 18],
                                            off_sb[0:18, j * P:(j + 1) * P],
                                            ident32[0:18, 0:18])
                    nc.scalar.copy(offt[:, tg * 432:(tg + 1) * 432], pso[:])

                # ---- phase C part 1 (DVE): base grid -> floors -> jT ----
                def st(tag, dt=f32):
                    return pc.tile([P, NT], dt, tag=tag, name=tag)

                def mt(tag, dt=f32):
                    return pc.tile([P, NF], dt, tag=tag, name=tag)

                posf = st("posf")
                nc.vector.tensor_scalar(posf[:], ramp128[:, 0:NT], 128.0,
                                        None, Alu.mult)
                nc.vector.tensor_scalar(posf[:], posf[:], pcol[:], None,
                                        Alu.add)
                q0i = st("q0i", i32)
                tmpq = st("tmpq")
                nc.vector.tensor_scalar(tmpq[:], posf[:], 1.0 / 96.0, None,
                                        Alu.mult)
                nc.vector.tensor_copy(q0i[:], tmpq[:])
                q0 = st("q0")
                nc.vector.tensor_copy(q0[:], q0i[:])
                r0 = st("r0")
                nc.vector.scalar_tensor_tensor(r0[:], q0[:], -96.0, posf[:],
                                               Alu.mult, Alu.add)
                ltz = st("ltz")
                nc.vector.tensor_scalar(ltz[:], r0[:], 0.0, None, Alu.is_lt)
                gez = st("gez")
                nc.vector.tensor_scalar(gez[:], r0[:], 96.0, None, Alu.is_ge)
                Rr = st("Rr")
                nc.vector.tensor_tensor(Rr[:], q0[:], ltz[:], Alu.subtract)
                nc.vector.tensor_tensor(Rr[:], Rr[:], gez[:], Alu.add)
                Cc = st("Cc")
                nc.vector.scalar_tensor_tensor(Cc[:], ltz[:], 96.0, r0[:],
                                               Alu.mult, Alu.add)
                nc.vector.scalar_tensor_tensor(Cc[:], gez[:], -96.0, Cc[:],
                                               Alu.mult, Alu.add)

                T1 = mt("T1")
                T2 = mt("T2")
                T3 = mt("T3")
                T4 = mt("T4")
                T5 = mt("T5")
                T6 = mt("T6")
                T7 = mt("T7")
                T8 = mt("T8")
                VI = mt("VI", i32)
                A0 = mt("A0", f16)
                A1 = mt("A1", f16)
                B0 = mt("B0", f16)
                B1 = mt("B1", f16)

                for k in range(K):
                    ky, kx = k // 3, k % 3
                    nc.vector.tensor_scalar(T1[:, k * NT:(k + 1) * NT],
                                            Rr[:], float(ky - 1), None,
                                            Alu.add)
                    nc.vector.tensor_scalar(T2[:, k * NT:(k + 1) * NT],
                                            Cc[:], float(kx - 1), None,
                                            Alu.add)
                offv = offt[:].rearrange("p (t pl) -> p pl t", pl=18)
                # py (T1), px (T2)
                nc.vector.tensor_tensor(
                    T1[:].rearrange("p (k t) -> p k t", k=K),
                    offv[:, 0:18:2, :],
                    T1[:].rearrange("p (k t) -> p k t", k=K), Alu.add)
                nc.vector.tensor_tensor(
                    T2[:].rearrange("p (k t) -> p k t", k=K),
                    offv[:, 1:18:2, :],
                    T2[:].rearrange("p (k t) -> p k t", k=K), Alu.add)

                def floor_frac(v, vf, fr, ng):
                    nc.vector.tensor_copy(VI[:], v[:])
                    nc.vector.tensor_copy(vf[:], VI[:])
                    nc.vector.tensor_tensor(fr[:], v[:], vf[:], Alu.subtract)
                    nc.vector.tensor_scalar(ng[:], fr[:], 0.0, None,
                                            Alu.is_lt)
                    nc.vector.tensor_tensor(vf[:], vf[:], ng[:],
                                            Alu.subtract)
                    nc.vector.tensor_tensor(fr[:], fr[:], ng[:], Alu.add)

                floor_frac(T1, T3, T4, T7)   # y0=T3, fy=T4
                floor_frac(T2, T5, T6, T7)   # x0=T5, fx=T6

                # jT = (clip(y0)+1)*98 + clip(x0)+1 (clips into fresh tiles:
                # the mask below still needs unclipped y0/x0)
                T3c = mt("T3c")
                T5c = mt("T5c")
                TJ = mt("TJ")
                nc.vector.tensor_scalar(T3c[:], T3[:], -1.0, 95.0, Alu.max,
                                        Alu.min)
                nc.vector.tensor_scalar(T5c[:], T5[:], -1.0, 95.0, Alu.max,
                                        Alu.min)
                nc.vector.tensor_scalar(T5c[:], T5c[:], 99.0, None, Alu.add)
                nc.vector.scalar_tensor_tensor(TJ[:], T3c[:], 98.0, T5c[:],
                                               Alu.mult, Alu.add)   # JT

                # ---- x: chunked load + pad/convert (Act), interleaved with
                # phase-A transpose groups as their input columns land ----
                dst = x16p[:, PW:PW + H * PW].rearrange(
                    "p (r w) -> p r w", w=PW)[:, :, 1:1 + W]
                GB4 = 4  # store-groups per evac batch
                b_ranges = [(0, 4), (4, 9), (9, 13), (13, 19)]

                def phase_a_groups(b0, b1):
                    # x_cp row j = [pix j | pix j+98]: transpose each group
                    # at +0 and +98 so rows are built whole in SBUF and the
                    # DMA writes contiguous 512B rows
                    for b in range(b0, b1):
                        ptb = ptP.tile([P, GB4 * 2 * P], f16, tag="ptb")
                        for g in range(GB4):
                            t = b * GB4 + g
                            nc.tensor.transpose(
                                ptb[:, (2 * g) * P:(2 * g + 1) * P],
                                x16p[:, t * P:(t + 1) * P], ident16[:])
                            nc.tensor.transpose(
                                ptb[:, (2 * g + 1) * P:(2 * g + 2) * P],
                                x16p[:, t * P + 98:t * P + 98 + P],
                                ident16[:])
                        evp = evpP.tile([P, GB4 * 2 * P], f16, tag="evp")
                        nc.scalar.copy(evp[:], ptb[:])
                        dstc = AP(tensor=_h(x_cp),
                                  offset=b * GB4 * P * 2 * P,
                                  ap=[[2 * P, P], [P * 2 * P, GB4],
                                      [1, 2 * P]])
                        nc.sync.dma_start(
                            dstc,
                            evp[:].rearrange("r (g c) -> r g c", g=GB4))

                XC = 4
                QW = NPOS // XC
                RQ = H // XC
                for q in range(XC):
                    x_sb = pld.tile([P, QW], f32, tag=f"xsb{q % 2}")
                    nc.sync.dma_start(x_sb[:],
                                      x_in[:, q * QW:(q + 1) * QW])
                    nc.scalar.copy(dst[:, q * RQ:(q + 1) * RQ, :],
                                   x_sb[:].rearrange("p (r w) -> p r w", w=W))
                    phase_a_groups(*b_ranges[q])
                    if q == 2:
                        # ---- phase D: indices to wrap-major DRAM, then one
                        # stride-0-source DMA reloads + replicates per tap ----
                        for k in range(K):
                            psi = pdp.tile([NT, P], f32, tag="psi")
                            nc.tensor.transpose(psi[:],
                                                TJ[:, k * NT:(k + 1) * NT],
                                                ident32[:])
                            evi0 = evip.tile([NT, P], i16, tag="evi0")
                            nc.vector.tensor_copy(evi0[:], psi[:])
                            # pre-permute on DVE so the wrap-major store has
                            # contiguous runs: evi[c, el*8+eh] = evi0[c, e]
                            # with e = eh*16+el
                            evi = evip.tile([NT, P], i16, tag="evi")
                            nc.vector.tensor_copy(
                                evi[:].rearrange("c (el eh) -> c el eh",
                                                 el=16, eh=8),
                                evi0[:].rearrange("c (eh el) -> c el eh",
                                                  eh=8, el=16))
                            # idx i=t*128+eh*16+el sits at evi f=el*8+eh;
                            # store to w = el*576 + 8t + eh (wrap-major)
                            dsti = AP(tensor=_h(idx_rows),
                                      offset=k * NPOS,
                                      ap=[[8, NT], [576, 16], [1, 8]])
                            srci = evi[:].rearrange(
                                "c (el eh) -> c el eh", el=16, eh=8)
                            nc.sync.dma_start(dsti, srci)
                            # reload wrapped + replicate to all 128
                            # partitions in ONE DMA (stride-0 src dim)
                            srcq = AP(tensor=_h(idx_rows), offset=k * NPOS,
                                      ap=[[0, 8], [576, 16], [1, 576]])
                            eng = nc.gpsimd if k == 0 else nc.sync
                            eng.dma_start(idxw[k][:], srcq)

                # ---- conv weights -> WkT ----
                w_sb = pld.tile([P, 1152], f32, tag="wsb")
                nc.sync.dma_start(w_sb[:], w_in[:])
                nc.scalar.copy(w16[:], w_sb[:])
                for k in range(K):
                    wkc = evip.tile([P, P], f16, tag="wkc")
                    nc.scalar.copy(wkc[:], w16[:, k:1152:9])
                    ptw = ptwP.tile([P, P], f16, tag="ptw")
                    nc.tensor.transpose(ptw[:], wkc[:], ident16[:])
                    nc.scalar.copy(WkT[:, k * P:(k + 1) * P], ptw[:])

                # ---- phase C part 2: mask + corner weights (DVE) ----
                # mask mm = (y0 in [-1,95]) & (x0 in [-1,95]) -> T8
                nc.vector.tensor_scalar(T8[:], T3[:], -1.0, None, Alu.is_ge)
                nc.vector.tensor_scalar(T1[:], T3[:], 95.0, None, Alu.is_le)
                nc.vector.tensor_tensor(T8[:], T8[:], T1[:], Alu.mult)
                nc.vector.tensor_scalar(T1[:], T5[:], -1.0, None, Alu.is_ge)
                nc.vector.tensor_scalar(T2[:], T5[:], 95.0, None, Alu.is_le)
                nc.vector.tensor_tensor(T1[:], T1[:], T2[:], Alu.mult)
                nc.vector.tensor_tensor(T8[:], T8[:], T1[:], Alu.mult)

                # wbot (T2) = fy*mm ; wtop (T1) = mm - wbot ; omfx (T7)
                nc.vector.tensor_tensor(T2[:], T4[:], T8[:], Alu.mult)
                nc.vector.tensor_tensor(T1[:], T8[:], T2[:], Alu.subtract)
                nc.vector.tensor_scalar(T7[:], T6[:], -1.0, 1.0, Alu.mult,
                                        Alu.add)
                nc.vector.tensor_tensor(A0[:], T1[:], T7[:], Alu.mult)
                nc.vector.tensor_tensor(A1[:], T1[:], T6[:], Alu.mult)
                nc.vector.tensor_tensor(B0[:], T2[:], T7[:], Alu.mult)
                nc.vector.tensor_tensor(B1[:], T2[:], T6[:], Alu.mult)

                # pair-duplicated slot-interleaved corner weights
                for s, tt_ in enumerate((A0, B0, A1, B1)):
                    dstD = AP(tensor=_h(awD), offset=awD[:].offset + 2 * s,
                              ap=[list(awD[:].ap[0]), [8, NF], [1, 2]])
                    srcD = AP(tensor=_h(tt_[:].tensor),
                              offset=tt_[:].offset,
                              ap=[list(tt_[:].ap[0]), [1, NF], [0, 2]])
                    nc.vector.tensor_copy(dstD, srcD)

            # ---------- phase F: main loop ----------
            xt_win = AP(tensor=_h(x_cp), offset=0,
                        ap=[[2 * P, NPADR - 1], [1, 4 * P]])
            with tc.tile_pool(name="g", bufs=3) as gp, \
                 tc.tile_pool(name="c4p", bufs=2) as c4p, \
                 tc.tile_pool(name="c4vp", bufs=3) as c4vp, \
                 tc.tile_pool(name="osp", bufs=2) as osp, \
                 tc.tile_pool(name="vpsP", bufs=2, space="PSUM") as vpsP, \
                 tc.tile_pool(name="outps", bufs=2, space="PSUM") as outps:
                iters = [(c, k) for c in range(NCH) for k in range(K)]

                def issue_gather(c, k):
                    # pos-major: out[p, jj, :] = quad of position
                    # (c*12 + h*6 + jj)*128 + p
                    g4 = gp.tile([P, JB, 4 * P], f16, tag="g4", name="g4")
                    for h in range(CW // GW):
                        i0 = (c * CW + h * GW) // 16
                        nc.gpsimd.dma_gather(
                            g4[:, h * 6:(h + 1) * 6, :], xt_win,
                            idxw[k][:, i0:i0 + GW // 16],
                            num_idxs=GW, num_idxs_reg=GW,
                            elem_size=4 * P, elem_step=2 * P, transpose=False)
                    return g4

                g4_next = issue_gather(*iters[0])
                for idx_it, (c, k) in enumerate(iters):
                    if k == 0:
                        out_ps = outps.tile([P, CW], f32, tag="ops",
                                            name="out_ps")
                    g4 = g4_next
                    if idx_it + 1 < len(iters):
                        g4_next = issue_gather(*iters[idx_it + 1])

                    # bilinear corner weighting: one DVE op, all 48 units;
                    # awD's stride-0 dim broadcasts each per-position weight
                    # along the 128 channels (last [1,2] dim keeps 2x mode)
                    c4 = c4p.tile([P, JB * 4, P], f16, tag="c4", name="c4")
                    aw_ap = AP(tensor=_h(awD),
                               offset=awD[:].offset + (k * NT + c * JB) * 8,
                               ap=[list(awD[:].ap[0]), [2, JB * 4],
                                   [0, P // 2], [1, 2]])
                    nc.vector.tensor_tensor(
                        c4[:], g4[:].rearrange("p j (s c) -> p (j s) c", c=P),
                        aw_ap, Alu.mult)

                    HB = 512  # one f32 PSUM bank; matmul out can't cross banks
                    for h in range(CW // HB):
                        vps = vpsP.tile([P, HB], f32, tag="vps")
                        for jj in range(HB // P):
                            u = (h * (HB // P) + jj) * 4
                            for s in range(4):
                                nc.tensor.matmul(
                                    vps[:, jj * P:(jj + 1) * P],
                                    c4[:, u + s, :], ident16[:],
                                    start=(s == 0), stop=(s == 3),
                                    skip_group_check=True)
                        c4v = c4vp.tile([P, HB], f16, tag="c4v")
                        nc.scalar.copy(c4v[:], vps[:])
                        nc.tensor.matmul(
                            out_ps[:, h * HB:(h + 1) * HB],
                            WkT[:, k * P:(k + 1) * P], c4v[:],
                            start=(k == 0), stop=(k == K - 1),
                            skip_group_check=True)
                        if k == K - 1:
                            # store each 512-block as soon as its
                            # accumulation group stops (short drain tail)
                            osb = osp.tile([P, HB], f16, tag="osb")
                            nc.scalar.copy(osb[:],
                                           out_ps[:, h * HB:(h + 1) * HB])
                            nc.sync.dma_start(
                                out[:, c * CW + h * HB:c * CW + (h + 1) * HB],
                                osb[:])
    nc.compile()
    return nc


_NC = None


def kernel(x, offset, weight):
    global _NC
    if _NC is None:
        _NC = build_nc()
    from concourse.bass_utils import run_bass_kernel_spmd
    B = x.shape[0]
    w2 = np.ascontiguousarray(np.asarray(weight).reshape(P, 1152)).astype(np.float32)
    in_maps = []
    for b in range(B):
        in_maps.append({
            "x": np.ascontiguousarray(np.asarray(x)[b].reshape(P, NPOS), dtype=np.float32),
            "offset": np.ascontiguousarray(np.asarray(offset)[b].reshape(18, NPOS), dtype=np.float32),
            "weight": w2,
        })
    res = run_bass_kernel_spmd(_NC, in_maps, list(range(B)))
    outs = [res.results[b]["out"].reshape(P, H, W) for b in range(B)]
    return np.stack(outs).astype(np.float32)
